# revision 1
# baseline (speedup 1.0000x reference)
import numpy as np

# Problem constants (nn_EnhancedMultiHeadAttention_3350074491572)
N, E, H, D = 4096, 512, 8, 64
SCALE = np.sqrt(np.float32(D)).astype(np.float32)


def _head_slices(Wq, bq, Wk, bk, Wv, bv, Wo):
    # Head h of Q = x @ Wq.T + bq takes rows h*D:(h+1)*D of Wq (and bq).
    rs = lambda W: W.reshape(H, D, E)
    # Partial of out = ctx @ Wo.T: head h contributes ctx_h @ (Wo.T)[h*D:(h+1)*D, :]
    Wo_h = np.ascontiguousarray(Wo.T).reshape(H, D, E)
    return (rs(Wq), bq.reshape(H, D), rs(Wk), bk.reshape(H, D),
            rs(Wv), bv.reshape(H, D), Wo_h)


def _numpy_head(x, Wq_h, bq_h, Wk_h, bk_h, Wv_h, bv_h, Wo_h):
    Q = x @ Wq_h.T + bq_h          # [N, D]
    K = x @ Wk_h.T + bk_h
    V = x @ Wv_h.T + bv_h
    s = (Q @ K.T) / SCALE          # [N, N]
    s = s - s.max(axis=-1, keepdims=True)
    e = np.exp(s, dtype=np.float32)
    a = e / e.sum(axis=-1, keepdims=True)
    ctx = a @ V                    # [N, D]
    out_p = ctx @ Wo_h             # [N, E] partial sum
    return out_p, a, Q, K, V


def _kernel_numpy(x, Wq, bq, Wk, bk, Wv, bv, Wo, bo):
    sl = _head_slices(Wq, bq, Wk, bk, Wv, bv, Wo)
    outs = [_numpy_head(x, *[t[h] for t in sl]) for h in range(H)]
    out = np.sum([o[0] for o in outs], axis=0).astype(np.float32) + bo
    attn = np.stack([o[1] for o in outs])                      # [H, N, N]
    Q = np.stack([o[2] for o in outs], axis=1)                 # [N, H, D]
    K = np.stack([o[3] for o in outs], axis=1)
    V = np.stack([o[4] for o in outs], axis=1)
    return (out.astype(np.float32), attn.astype(np.float32),
            Q.astype(np.float32), K.astype(np.float32), V.astype(np.float32))


def _kernel_jax(x, Wq, bq, Wk, bk, Wv, bv, Wo, bo):
    import jax
    import jax.numpy as jnp

    devs = jax.devices()
    if len(devs) < H:
        raise RuntimeError(f"need {H} devices, have {len(devs)}")

    def head_fn(x, Wq_h, bq_h, Wk_h, bk_h, Wv_h, bv_h, Wo_h):
        Q = x @ Wq_h.T + bq_h                  # [N, D]
        K = x @ Wk_h.T + bk_h
        V = x @ Wv_h.T + bv_h
        s = (Q @ K.T) / SCALE                  # [N, N]
        a = jax.nn.softmax(s, axis=-1)
        ctx = a @ V                            # [N, D]
        out_p = ctx @ Wo_h                     # [N, E]
        return out_p, a, Q, K, V

    f = jax.pmap(head_fn, in_axes=(None, 0, 0, 0, 0, 0, 0, 0),
                 devices=devs[:H])
    sl = _head_slices(Wq, bq, Wk, bk, Wv, bv, Wo)
    out_p, a, Q, K, V = f(jnp.asarray(x), *[jnp.asarray(t) for t in sl])
    out = np.asarray(out_p).sum(axis=0).astype(np.float32) + bo
    attn = np.asarray(a)                       # [H, N, N]
    Qf = np.asarray(Q).transpose(1, 0, 2)      # [H,N,D] -> [N,H,D]
    Kf = np.asarray(K).transpose(1, 0, 2)
    Vf = np.asarray(V).transpose(1, 0, 2)
    return (np.ascontiguousarray(out, dtype=np.float32),
            np.ascontiguousarray(attn, dtype=np.float32),
            np.ascontiguousarray(Qf, dtype=np.float32),
            np.ascontiguousarray(Kf, dtype=np.float32),
            np.ascontiguousarray(Vf, dtype=np.float32))


def kernel(x, Wq, bq, Wk, bk, Wv, bv, Wo, bo):
    args = [np.asarray(t, dtype=np.float32) for t in
            (x, Wq, bq, Wk, bk, Wv, bv, Wo, bo)]
    try:
        return _kernel_jax(*args)
    except Exception:
        return _kernel_numpy(*args)
